# revision 12
# baseline (speedup 1.0000x reference)
"""Trainium2 Bass kernel for attention-pooled BiLSTM + GRU decoder (ragged seq).

Strategy: data-parallel over batch B=128 across 8 cores (16 batches/core).
Per core:
  - attention-normalize A, attention-pool features -> C (einsum on PE)
  - BiLSTM over T=25 (x-part precomputed as big matmul; recurrent part
    stepwise, gates on partitions, batch on the free dim)
  - GRU decode over T=25 (same layout; teacher-forced inputs precomputed)
  - generator matmul -> logits
Host: input layout prep (transposes/casts/gate reorder), embedding lookup,
final ragged gather (output shape depends on text_length values).
"""

import contextlib

import numpy as np
import ml_dtypes

import concourse.bass as bass
import concourse.tile as tile
import concourse.mybir as mybir
from concourse import bacc
from concourse.bass_utils import run_bass_kernel_spmd

# problem shapes (hardcoded per spec)
B, T, NCH, NCLASS = 128, 25, 512, 38
HWS = 256           # 8*32 spatial
HL = 256            # LSTM hidden per direction
GL = 4 * HL         # 1024 LSTM gates
HG = NCH            # 512 GRU hidden
GG = 3 * HG         # 1536 GRU gates
NCORES = 8
BC = B // NCORES    # 16 batches per core
TB = T * BC         # 400 columns, t-major: col = t*BC + b

F32 = mybir.dt.float32
BF16 = mybir.dt.bfloat16
AF = mybir.ActivationFunctionType
BF16_NP = ml_dtypes.bfloat16


def build_program(repeat: int = 0, phases: int = 7):
    nc = bacc.Bacc("TRN2", target_bir_lowering=False, debug=False,
                   num_devices=NCORES)

    def din(name, shape, dt=BF16):
        return nc.dram_tensor(name, list(shape), dt, kind="ExternalInput")

    dram = {}
    dram["featT"] = din("featT", (128, 2, BC, NCH))        # (p, hw_kt, b, c)
    dram["A_t"] = din("A_t", (128, 2, BC, T), F32)         # (p, hw_kt, b, t)
    dram["embT"] = din("embT", (128, 4, TB))               # (p, c_kt, t*BC+b)
    for nm in ["wih_f", "wih_b"]:
        dram[nm] = din(nm, (128, 4, GL))
    for nm in ["whh_f", "whh_b"]:
        dram[nm] = din(nm, (128, 2, GL))
    dram["wihg"] = din("wihg", (128, 8, GG))
    dram["whhg"] = din("whhg", (128, 4, GG))
    dram["wgen"] = din("wgen", (128, 4, NCLASS))
    dram["bias_f"] = din("bias_f", (128, 8), F32)
    dram["bias_b"] = din("bias_b", (128, 8), F32)
    dram["bias_g"] = din("bias_g", (128, 12), F32)
    dram["bhhn"] = din("bhhn", (128, 128))
    dram["bias_gen"] = din("bias_gen", (NCLASS, 1), F32)
    dram["ident"] = din("ident", (128, 128))
    dram["emt"] = din("emt", (128, 4, BC))   # one-hot: [k,mt,b]=(k==mt)
    dram["attn_o"] = nc.dram_tensor("attn_o", [128, 2, BC, T], F32,
                                    kind="ExternalOutput")
    dram["logits_o"] = nc.dram_tensor("logits_o", [NCLASS, TB], F32,
                                      kind="ExternalOutput")

    with tile.TileContext(nc) as tc:
        with (
            tc.tile_pool(name="wpool", bufs=1) as wpool,
            tc.tile_pool(name="data", bufs=1) as data,
            tc.tile_pool(name="state", bufs=1) as state,
            tc.tile_pool(name="ew", bufs=3) as ew,
            tc.tile_pool(name="ps", bufs=1, space="PSUM") as ps,
        ):
            pools = (wpool, data, state, ew, ps)
            loop_cm = (tc.For_i(0, repeat, 1) if repeat
                       else contextlib.nullcontext())
            with loop_cm:
                _emit(nc, phases, pools, dram)

    nc.compile()
    return nc


def _emit(nc, phases, pools, dram):
    wpool, data, state, ew, ps = pools

    # ---- resident weights / constants ----
    w_feat = wpool.tile([128, 2, BC, NCH], BF16, name="w_feat")
    w_A = wpool.tile([128, 2, BC, T], F32, name="w_A")
    w_wih = [wpool.tile([128, 4, GL], BF16, tag=f"wih{d}", name=f"w_wih{d}")
             for d in range(2)]
    w_whh = [wpool.tile([128, 2, GL], BF16, tag=f"whh{d}", name=f"w_whh{d}")
             for d in range(2)]
    w_wihg = wpool.tile([128, 8, GG], BF16, name="w_wihg")
    w_whhg = wpool.tile([128, 4, GG], BF16, name="w_whhg")
    w_wgen = wpool.tile([128, 4, NCLASS], BF16, name="w_wgen")
    w_bias = [wpool.tile([128, 8], F32, tag=f"bias{d}", name=f"w_bias{d}")
              for d in range(2)]
    w_biasg = wpool.tile([128, 12], F32, name="w_biasg")
    w_bhhn = wpool.tile([128, 128], BF16, name="w_bhhn")
    w_bgen = wpool.tile([NCLASS, 1], F32, name="w_bgen")
    w_I = wpool.tile([128, 128], BF16, name="w_I")
    w_emt = wpool.tile([128, 4, BC], BF16, name="w_emt")

    nc.sync.dma_start(out=w_A, in_=dram["A_t"].ap())
    nc.sync.dma_start(out=w_feat, in_=dram["featT"].ap())
    nc.sync.dma_start(out=w_wih[0], in_=dram["wih_f"].ap())
    nc.sync.dma_start(out=w_wih[1], in_=dram["wih_b"].ap())
    nc.sync.dma_start(out=w_whh[0], in_=dram["whh_f"].ap())
    nc.sync.dma_start(out=w_whh[1], in_=dram["whh_b"].ap())
    nc.sync.dma_start(out=w_bias[0], in_=dram["bias_f"].ap())
    nc.sync.dma_start(out=w_bias[1], in_=dram["bias_b"].ap())
    nc.sync.dma_start(out=w_I, in_=dram["ident"].ap())
    nc.sync.dma_start(out=w_emt, in_=dram["emt"].ap())
    nc.sync.dma_start(out=w_bhhn, in_=dram["bhhn"].ap())
    nc.sync.dma_start(out=w_wihg, in_=dram["wihg"].ap())
    nc.sync.dma_start(out=w_whhg, in_=dram["whhg"].ap())
    nc.sync.dma_start(out=w_biasg, in_=dram["bias_g"].ap())
    nc.sync.dma_start(out=w_wgen, in_=dram["wgen"].ap())
    nc.sync.dma_start(out=w_bgen, in_=dram["bias_gen"].ap())

    # ---- big activations ----
    an_bf = data.tile([128, 2, BC, T], BF16, name="an_bf")
    ct_sb = data.tile([128, 4, TB], BF16, name="ct_sb")
    lstm_out = data.tile([128, 4, TB], BF16, name="lstm_out")
    xe = [data.tile([128, TB], BF16, tag=f"xe{k}", name=f"xe{k}")
          for k in range(4)]
    xpre = [data.tile([128, 8, TB], BF16, tag=f"xpre{d}", name=f"xpre{d}")
            for d in range(2)]
    gi_emb = data.tile([128, 12, TB], F32, name="gi_emb")
    gi = data.tile([128, 12, TB], BF16, name="gi")
    hg = data.tile([128, 4, TB], BF16, name="hg")
    attn_f32 = data.tile([128, 2, BC, T], F32, name="attn_f32")
    logits_sb = data.tile([NCLASS, TB], F32, name="logits_sb")

    for j in range(4):
        nc.sync.dma_start(out=xe[j], in_=dram["embT"].ap()[:, j, :])

    # constants / states
    ones_col = state.tile([128, 1], F32, name="ones_col")
    ones_row = state.tile([1, 128], F32, name="ones_row")
    zer_rhs = state.tile([128, BC], BF16, name="zer_rhs")
    zer_h4 = state.tile([128, 4, BC], BF16, name="zer_h4")
    lstm_c = state.tile([128, 2, 2, BC], F32, name="lstm_c")
    nc.vector.memset(ones_col, 1.0)
    nc.vector.memset(ones_row, 1.0)
    nc.vector.memset(zer_rhs, 0.0)
    nc.vector.memset(zer_h4, 0.0)
    nc.vector.memset(lstm_c, 0.0)

    # ============ phase 1: normalize A ============
    ps_sum = ps.tile([1, TB], F32, tag="pA", bufs=2, name="ps_sum")
    for kt in range(2):
        nc.tensor.matmul(ps_sum, ones_col,
                         w_A[:, kt].rearrange("p b t -> p (b t)"),
                         start=(kt == 0), stop=(kt == 1))
    recip = ew.tile([1, TB], F32, name="recip")
    nc.vector.reciprocal(recip, ps_sum)
    ps_bc = ps.tile([128, BC, T], F32, tag="pA", bufs=2, name="ps_bc")
    nc.tensor.matmul(ps_bc.rearrange("p b t -> p (b t)"), ones_row, recip,
                     start=True, stop=True)
    for kt in range(2):
        nc.vector.tensor_mul(an_bf[:, kt], w_A[:, kt], ps_bc)
        nc.vector.tensor_mul(attn_f32[:, kt], w_A[:, kt], ps_bc)
    nc.sync.dma_start(out=dram["attn_o"].ap(), in_=attn_f32)
    if phases < 2:
        return

    # ============ phase 2: attention pooling -> ct_sb ============
    for ct in range(4):
        ps_c = ps.tile([128, BC, T], F32, tag="pA", bufs=2, name="ps_c")
        for b in range(BC):
            for kt in range(2):
                nc.tensor.matmul(
                    ps_c[:, b, :],
                    w_feat[:, kt, b, ct * 128:(ct + 1) * 128],
                    an_bf[:, kt, b, :],
                    start=(b == 0 and kt == 0),
                    stop=(b == BC - 1 and kt == 1))
        nc.vector.tensor_copy(
            ct_sb[:, ct].rearrange("p (t b) -> p b t", b=BC), ps_c)
    if phases < 3:
        return

    # ============ phase 3: LSTM x-precompute ============
    for d in range(2):
        for mt in range(8):
            ps_x = ps.tile([128, TB], F32, tag="pX", bufs=2, name="ps_x")
            for kt in range(4):
                nc.tensor.matmul(
                    ps_x, w_wih[d][:, kt, mt * 128:(mt + 1) * 128],
                    ct_sb[:, kt], start=(kt == 0), stop=(kt == 3))
            nc.scalar.activation(xpre[d][:, mt], ps_x, AF.Identity,
                                 bias=w_bias[d][:, mt:mt + 1])
    if phases < 4:
        return

    # ============ phase 4: BiLSTM recurrence ============
    # gate order (host-permuted): [i, f, o, g]; mt 0..5 sigmoid, 6..7 tanh
    # fwd (d=0) and bwd (d=1) run merged: one psum bank, shared elementwise
    for s in range(T):
        tcs = [s * BC, (T - 1 - s) * BC]          # col offset per dir
        ps_g = ps.tile([128, 2, 8, BC], F32, tag="psg", bufs=3, name="ps_g")
        for d in range(2):
            nc.tensor.matmul(ps_g[:, d], w_I,
                             xpre[d][:, :, tcs[d]:tcs[d] + BC],
                             start=(d == 0), stop=False)
        for d in range(2):
            for kt in range(2):
                if s == 0:
                    rhs = zer_rhs
                else:
                    tp = (s - 1) if d == 0 else (T - s)
                    rhs = lstm_out[:, 2 * d + kt, tp * BC:(tp + 1) * BC]
                for mt in range(8):
                    nc.tensor.matmul(
                        ps_g[:, d, mt],
                        w_whh[d][:, kt, mt * 128:(mt + 1) * 128],
                        rhs, start=False,
                        stop=(d == 1 and kt == 1 and mt == 7))
        sig = ew.tile([128, 2, 6, BC], F32, tag="sig", name="sig")
        tg = ew.tile([128, 2, 2, BC], F32, tag="tg", name="tg")
        nc.scalar.activation(sig, ps_g[:, :, 0:6], AF.Sigmoid)
        nc.scalar.activation(tg, ps_g[:, :, 6:8], AF.Tanh)
        tmp = ew.tile([128, 2, 2, BC], F32, tag="tmp", name="tmp")
        nc.vector.tensor_mul(lstm_c, lstm_c, sig[:, :, 2:4])
        nc.vector.tensor_mul(tmp, sig[:, :, 0:2], tg)
        nc.vector.tensor_add(lstm_c, lstm_c, tmp)
        tanhc = ew.tile([128, 2, 2, BC], F32, tag="tanhc", name="tanhc")
        nc.scalar.activation(tanhc, lstm_c, AF.Tanh)
        for d in range(2):
            nc.vector.tensor_mul(
                lstm_out[:, 2 * d:2 * d + 2, tcs[d]:tcs[d] + BC],
                sig[:, d, 4:6], tanhc[:, d])
    if phases < 5:
        return

    # ============ phase 5: GRU gi precompute ============
    # emb half: independent of the LSTM -> fills PE gaps during phase 4
    for mt in range(12):
        ps_ge = ps.tile([128, TB], F32, tag="pX", bufs=2, name="ps_ge")
        for kt in range(4):
            nc.tensor.matmul(ps_ge, w_wihg[:, 4 + kt, mt * 128:(mt + 1) * 128],
                             xe[kt], start=(kt == 0), stop=(kt == 3))
        nc.scalar.activation(gi_emb[:, mt], ps_ge, AF.Identity,
                             bias=w_biasg[:, mt:mt + 1])
    # lstm half + combine
    for mt in range(12):
        ps_gi = ps.tile([128, TB], F32, tag="pX", bufs=2, name="ps_gi")
        for kt in range(4):
            nc.tensor.matmul(ps_gi, w_wihg[:, kt, mt * 128:(mt + 1) * 128],
                             lstm_out[:, kt], start=(kt == 0), stop=(kt == 3))
        nc.vector.scalar_tensor_tensor(
            out=gi[:, mt], in0=ps_gi, scalar=0.0, in1=gi_emb[:, mt],
            op0=mybir.AluOpType.bypass, op1=mybir.AluOpType.add)
    if phases < 6:
        return

    # ============ phase 6: GRU recurrence ============
    # weight rows: r = 0:512, z = 512:1024, n = 1024:1536
    for t in range(T):
        tc0, tc1 = t * BC, (t + 1) * BC
        hprev = zer_h4 if t == 0 else hg[:, :, tc0 - BC:tc0]
        ps_r = ps.tile([128, 4, BC], F32, tag="psg", bufs=3, name="ps_r")
        ps_z = ps.tile([128, 4, BC], F32, tag="psg", bufs=3, name="ps_z")
        ps_n = ps.tile([128, 4, BC], F32, tag="psn", bufs=1, name="ps_n")
        nc.tensor.matmul(ps_r, w_I, gi[:, 0:4, tc0:tc1],
                         start=True, stop=False)
        nc.tensor.matmul(ps_z, w_I, gi[:, 4:8, tc0:tc1],
                         start=True, stop=False)
        nc.tensor.matmul(ps_n, w_bhhn, w_emt, start=True, stop=False)
        for kt in range(4):
            rhs = zer_rhs if t == 0 else hg[:, kt, tc0 - BC:tc0]
            for mt in range(4):
                nc.tensor.matmul(
                    ps_r[:, mt], w_whhg[:, kt, mt * 128:(mt + 1) * 128],
                    rhs, start=False, stop=(kt == 3 and mt == 3))
            for mt in range(4):
                nc.tensor.matmul(
                    ps_n[:, mt],
                    w_whhg[:, kt, 1024 + mt * 128:1024 + (mt + 1) * 128],
                    rhs, start=False, stop=(kt == 3 and mt == 3))
            for mt in range(4):
                nc.tensor.matmul(
                    ps_z[:, mt],
                    w_whhg[:, kt, 512 + mt * 128:512 + (mt + 1) * 128],
                    rhs, start=False, stop=(kt == 3 and mt == 3))
        r_t = ew.tile([128, 4, BC], F32, tag="r_t", name="r_t")
        z_t = ew.tile([128, 4, BC], F32, tag="z_t", name="z_t")
        zc_t = ew.tile([128, 4, BC], F32, tag="zc_t", name="zc_t")
        nc.scalar.activation(r_t, ps_r, AF.Sigmoid)
        nc.scalar.activation(z_t, ps_z, AF.Sigmoid)
        nc.scalar.activation(zc_t, ps_z, AF.Sigmoid, scale=-1.0)
        p1 = ew.tile([128, 4, BC], F32, tag="p1", name="p1")
        nc.vector.tensor_mul(p1, hprev, z_t)
        nt = ew.tile([128, 4, BC], F32, tag="nt", name="nt")
        nc.vector.tensor_mul(nt, ps_n, r_t)
        nc.vector.tensor_add(nt, nt, gi[:, 8:12, tc0:tc1])
        ntt = ew.tile([128, 4, BC], F32, tag="ntt", name="ntt")
        nc.scalar.activation(ntt, nt, AF.Tanh)
        nc.vector.tensor_mul(ntt, ntt, zc_t)
        nc.vector.tensor_add(hg[:, :, tc0:tc1], p1, ntt)
    if phases < 7:
        return

    # ============ phase 7: generator ============
    ps_l = ps.tile([NCLASS, TB], F32, tag="pX", bufs=2, name="ps_l")
    for kt in range(4):
        nc.tensor.matmul(ps_l, w_wgen[:, kt], hg[:, kt],
                         start=(kt == 0), stop=(kt == 3))
    nc.scalar.activation(logits_sb, ps_l, AF.Identity, bias=w_bgen)
    nc.sync.dma_start(out=dram["logits_o"].ap(), in_=logits_sb)


def prep_inputs(inputs):
    """Host-side: shard + lay out all tensors exactly as the SBUF tiles expect."""
    feature = np.asarray(inputs["feature"], np.float32)
    A = np.asarray(inputs["A"], np.float32)
    text = np.asarray(inputs["text"])
    char_emb = np.asarray(inputs["char_emb"], np.float32)

    def as_bf(x):
        return np.ascontiguousarray(x.astype(BF16_NP))

    # LSTM gate permutation i,f,g,o -> i,f,o,g
    perm = np.concatenate([np.arange(0, 512), np.arange(768, 1024),
                           np.arange(512, 768)])

    def lhsT_tiles(w):  # (K, M) -> (128, K//128, M)
        K, M = w.shape
        return np.ascontiguousarray(
            w.reshape(K // 128, 128, M).transpose(1, 0, 2))

    def lstm_w(d):
        sfx = "f" if d == 0 else "b"
        wih = np.asarray(inputs[f"W_ih_{sfx}"], np.float32)[perm]
        whh = np.asarray(inputs[f"W_hh_{sfx}"], np.float32)[perm]
        bias = (np.asarray(inputs[f"b_ih_{sfx}"], np.float32)
                + np.asarray(inputs[f"b_hh_{sfx}"], np.float32))[perm]
        return (as_bf(lhsT_tiles(wih.T)), as_bf(lhsT_tiles(whh.T)),
                np.ascontiguousarray(bias.reshape(8, 128).T))

    wihf, whhf, biasf = lstm_w(0)
    wihb, whhb, biasb = lstm_w(1)

    wihg_np = np.asarray(inputs["W_ih_g"], np.float32)
    whhg_np = np.asarray(inputs["W_hh_g"], np.float32)
    bihg = np.asarray(inputs["b_ih_g"], np.float32)
    bhhg = np.asarray(inputs["b_hh_g"], np.float32)
    biasg = bihg + np.concatenate([bhhg[:1024], np.zeros(512, np.float32)])
    bhhn_np = bhhg[1024:]
    bhhn_tiles = np.zeros((128, 128), np.float32)
    bhhn_tiles[:4] = bhhn_np.reshape(4, 128)
    wgen_np = np.asarray(inputs["W_gen"], np.float32)
    bgen = np.asarray(inputs["b_gen"], np.float32)

    emt = np.zeros((128, 4, BC), np.float32)
    for _mt in range(4):
        emt[_mt, _mt, :] = 1.0

    shared = {
        "wih_f": wihf, "wih_b": wihb, "whh_f": whhf, "whh_b": whhb,
        "wihg": as_bf(lhsT_tiles(wihg_np.T)),
        "whhg": as_bf(lhsT_tiles(whhg_np.T)),
        "wgen": as_bf(lhsT_tiles(wgen_np.T)),
        "bias_f": biasf, "bias_b": biasb,
        "bias_g": np.ascontiguousarray(biasg.reshape(12, 128).T),
        "bhhn": as_bf(bhhn_tiles),
        "bias_gen": np.ascontiguousarray(bgen.reshape(NCLASS, 1)),
        "ident": as_bf(np.eye(128, dtype=np.float32)),
        "emt": as_bf(emt),
    }

    prev_tok = np.concatenate(
        [np.zeros((B, 1), text.dtype), text[:, :-1]], axis=1)
    emb = char_emb[prev_tok]                                       # (B,T,512)

    in_maps = []
    for k in range(NCORES):
        sl = slice(k * BC, (k + 1) * BC)
        fk = feature[sl].reshape(BC, NCH, HWS)
        featT = fk.transpose(2, 0, 1).reshape(2, 128, BC, NCH)
        featT = np.ascontiguousarray(featT.transpose(1, 0, 2, 3))
        ak = A[sl].reshape(BC, T, HWS)
        A_tk = ak.transpose(2, 0, 1).reshape(2, 128, BC, T)
        A_tk = np.ascontiguousarray(A_tk.transpose(1, 0, 2, 3))
        ek = emb[sl]
        embT = ek.transpose(2, 1, 0).reshape(4, 128, T * BC)
        embT = np.ascontiguousarray(embT.transpose(1, 0, 2))
        m = dict(shared)
        m["featT"] = as_bf(featT)
        m["A_t"] = A_tk.astype(np.float32)
        m["embT"] = as_bf(embT)
        in_maps.append(m)
    return in_maps


def postprocess(results, text_length):
    logits = np.zeros((T, B, NCLASS), np.float32)
    attns = np.zeros((B, T, HWS), np.float32)
    for k, r in enumerate(results):
        sl = slice(k * BC, (k + 1) * BC)
        lo = r["logits_o"].reshape(NCLASS, T, BC)
        logits[:, sl, :] = lo.transpose(1, 2, 0)
        at = r["attn_o"]
        attns[sl] = at.transpose(2, 3, 1, 0).reshape(BC, T, HWS)
    lens = np.asarray(text_length)
    b_idx = np.repeat(np.arange(B), lens)
    t_idx = np.concatenate([np.arange(l) for l in lens])
    out_res = logits[t_idx, b_idx]
    out_attns = attns[b_idx, t_idx].reshape(-1, 8, 32)
    return out_res, out_attns


_CACHED = {}


def get_program(repeat: int = 0, phases: int = 7):
    key = (repeat, phases)
    if key not in _CACHED:
        _CACHED[key] = build_program(repeat, phases)
    return _CACHED[key]


def kernel(**inputs):
    nc = get_program(0)
    in_maps = prep_inputs(inputs)
    res = run_bass_kernel_spmd(nc, in_maps, core_ids=list(range(NCORES)))
    return postprocess(res.results, inputs["text_length"])


def _pjrt_runner(nc):
    """Persistent jitted SPMD callable (for repeated timed runs)."""
    import jax
    from jax.sharding import Mesh, PartitionSpec
    from jax.experimental.shard_map import shard_map
    from concourse.bass2jax import (_bass_exec_p, install_neuronx_cc_hook,
                                    partition_id_tensor)

    install_neuronx_cc_hook()
    pname = nc.partition_id_tensor.name if nc.partition_id_tensor else None
    in_names, out_names, out_avals, zero_outs = [], [], [], []
    for alloc in nc.m.functions[0].allocations:
        if not isinstance(alloc, mybir.MemoryLocationSet):
            continue
        name = alloc.memorylocations[0].name
        if alloc.kind == "ExternalInput":
            if name != pname:
                in_names.append(name)
        elif alloc.kind == "ExternalOutput":
            out_names.append(name)
            shape = tuple(alloc.tensor_shape)
            dtype = mybir.dt.np(alloc.dtype)
            out_avals.append(jax.core.ShapedArray(shape, dtype))
            zero_outs.append(np.zeros(shape, dtype))
    n_params, n_outs = len(in_names), len(out_avals)
    all_names = list(in_names) + list(out_names)
    if pname is not None:
        all_names.append(pname)

    def _body(*args):
        operands = list(args)
        if pname is not None:
            operands.append(partition_id_tensor())
        return tuple(_bass_exec_p.bind(
            *operands, out_avals=tuple(out_avals), in_names=tuple(all_names),
            out_names=tuple(out_names), lowering_input_output_aliases=(),
            sim_require_finite=True, sim_require_nnan=True, nc=nc))

    devices = jax.devices()[:NCORES]
    mesh = Mesh(np.asarray(devices), ("core",))
    fn = jax.jit(
        shard_map(_body, mesh=mesh,
                  in_specs=(PartitionSpec("core"),) * (n_params + n_outs),
                  out_specs=(PartitionSpec("core"),) * n_outs,
                  check_rep=False),
        keep_unused=True)
    return fn, in_names, zero_outs


def _timed_wall(nc, in_maps, reps):
    import time
    import jax
    fn, in_names, zero_outs = _pjrt_runner(nc)
    concat_in = [np.concatenate([m[n] for m in in_maps], axis=0)
                 for n in in_names]
    concat_z = [np.concatenate([z] * NCORES, axis=0) for z in zero_outs]
    jax.block_until_ready(fn(*concat_in, *concat_z))  # warm compile
    best = float("inf")
    for _ in range(reps):
        t0 = time.perf_counter()
        jax.block_until_ready(fn(*concat_in, *concat_z))
        best = min(best, time.perf_counter() - t0)
    return best


def measure_hw_time(inputs, reps=5, r0=1, r1=129, phases=7):
    """Per-iteration device time via repeat-count delta (cancels dispatch cost)."""
    in_maps = prep_inputs(inputs)
    w0 = _timed_wall(get_program(r0, phases), in_maps, reps)
    w1 = _timed_wall(get_program(r1, phases), in_maps, reps)
    return (w1 - w0) / (r1 - r0) * 1e9


# revision 19
# speedup vs baseline: 4.1643x; 4.1643x over previous
"""Trainium2 Bass kernel for attention-pooled BiLSTM + GRU decoder (ragged seq).

Strategy: data-parallel over batch B=128 across 8 cores (16 batches/core).
Per core:
  - attention-normalize A, attention-pool features -> C (einsum on PE)
  - BiLSTM over T=25 (x-part precomputed as big matmul; recurrent part
    stepwise, gates on partitions, batch on the free dim)
  - GRU decode over T=25 (same layout; teacher-forced inputs precomputed)
  - generator matmul -> logits
Host: input layout prep (transposes/casts/gate reorder), embedding lookup,
final ragged gather (output shape depends on text_length values).
"""

import contextlib

import numpy as np
import ml_dtypes

import concourse.bass as bass
import concourse.tile as tile
import concourse.mybir as mybir
from concourse import bacc
from concourse.bass_utils import run_bass_kernel_spmd

# problem shapes (hardcoded per spec)
B, T, NCH, NCLASS = 128, 25, 512, 38
HWS = 256           # 8*32 spatial
HL = 256            # LSTM hidden per direction
GL = 4 * HL         # 1024 LSTM gates
HG = NCH            # 512 GRU hidden
GG = 3 * HG         # 1536 GRU gates
NCORES = 8
BC = B // NCORES    # 16 batches per core
TB = T * BC         # 400 columns, t-major: col = t*BC + b

F32 = mybir.dt.float32
BF16 = mybir.dt.bfloat16
AF = mybir.ActivationFunctionType
BF16_NP = ml_dtypes.bfloat16


def build_program(repeat: int = 0, phases: int = 7):
    nc = bacc.Bacc("TRN2", target_bir_lowering=False, debug=False,
                   num_devices=NCORES)

    def din(name, shape, dt=BF16):
        return nc.dram_tensor(name, list(shape), dt, kind="ExternalInput")

    dram = {}
    dram["featT"] = din("featT", (128, 2, BC, NCH))        # (p, hw_kt, b, c)
    dram["A_t"] = din("A_t", (128, 2, BC, T), F32)         # (p, hw_kt, b, t)
    dram["embT"] = din("embT", (128, 4, TB))               # (p, c_kt, t*BC+b)
    for nm in ["wih_f", "wih_b"]:
        dram[nm] = din(nm, (128, 4, GL))
    for nm in ["whh_f", "whh_b"]:
        dram[nm] = din(nm, (128, 2, GL))
    dram["wihg"] = din("wihg", (128, 8, GG))
    dram["whhg"] = din("whhg", (128, 4, GG))
    dram["wgen"] = din("wgen", (128, 4, NCLASS))
    dram["bias_f"] = din("bias_f", (128, 8), F32)
    dram["bias_b"] = din("bias_b", (128, 8), F32)
    dram["bias_g"] = din("bias_g", (128, 12), F32)
    dram["bhhn"] = din("bhhn", (128, 128))
    dram["bias_gen"] = din("bias_gen", (NCLASS, 1), F32)
    dram["ident"] = din("ident", (128, 128))
    dram["emt"] = din("emt", (128, 4, BC))   # one-hot: [k,mt,b]=(k==mt)
    dram["attn_o"] = nc.dram_tensor("attn_o", [128, 2, BC, T], F32,
                                    kind="ExternalOutput")
    dram["logits_o"] = nc.dram_tensor("logits_o", [NCLASS, TB], F32,
                                      kind="ExternalOutput")

    with tile.TileContext(nc) as tc:
        with (
            tc.tile_pool(name="wpool", bufs=1) as wpool,
            tc.tile_pool(name="data", bufs=1) as data,
            tc.tile_pool(name="state", bufs=1) as state,
            tc.tile_pool(name="ew", bufs=4) as ew,
            tc.tile_pool(name="ps", bufs=1, space="PSUM") as ps,
        ):
            pools = (wpool, data, state, ew, ps)
            loop_cm = (tc.For_i(0, repeat, 1) if repeat
                       else contextlib.nullcontext())
            with loop_cm:
                _emit(nc, phases, pools, dram)

    nc.compile()
    return nc


def _emit(nc, phases, pools, dram):
    wpool, data, state, ew, ps = pools

    # ---- resident weights / constants ----
    w_feat = wpool.tile([128, 2, BC, NCH], BF16, name="w_feat")
    w_A = wpool.tile([128, 2, BC, T], F32, name="w_A")
    w_wih = [wpool.tile([128, 4, GL], BF16, tag=f"wih{d}", name=f"w_wih{d}")
             for d in range(2)]
    w_whh = [wpool.tile([128, 2, GL], BF16, tag=f"whh{d}", name=f"w_whh{d}")
             for d in range(2)]
    w_wihg = wpool.tile([128, 8, GG], BF16, name="w_wihg")
    w_whhg = wpool.tile([128, 4, GG], BF16, name="w_whhg")
    w_wgen = wpool.tile([128, 4, NCLASS], BF16, name="w_wgen")
    w_bias = [wpool.tile([128, 8], F32, tag=f"bias{d}", name=f"w_bias{d}")
              for d in range(2)]
    w_biasg = wpool.tile([128, 12], F32, name="w_biasg")
    w_bhhn = wpool.tile([128, 128], BF16, name="w_bhhn")
    w_bgen = wpool.tile([NCLASS, 1], F32, name="w_bgen")
    w_I = wpool.tile([128, 128], BF16, name="w_I")
    w_emt = wpool.tile([128, 4, BC], BF16, name="w_emt")

    nc.sync.dma_start(out=w_A, in_=dram["A_t"].ap())
    nc.sync.dma_start(out=w_feat[:, 0], in_=dram["featT"].ap()[:, 0])
    nc.sync.dma_start(out=w_feat[:, 1], in_=dram["featT"].ap()[:, 1])
    nc.sync.dma_start(out=w_wih[0], in_=dram["wih_f"].ap())
    nc.sync.dma_start(out=w_wih[1], in_=dram["wih_b"].ap())
    nc.sync.dma_start(out=w_whh[0], in_=dram["whh_f"].ap())
    nc.sync.dma_start(out=w_whh[1], in_=dram["whh_b"].ap())
    nc.sync.dma_start(out=w_bias[0], in_=dram["bias_f"].ap())
    nc.sync.dma_start(out=w_bias[1], in_=dram["bias_b"].ap())
    nc.sync.dma_start(out=w_I, in_=dram["ident"].ap())
    nc.sync.dma_start(out=w_emt, in_=dram["emt"].ap())
    nc.sync.dma_start(out=w_bhhn, in_=dram["bhhn"].ap())
    nc.sync.dma_start(out=w_wihg, in_=dram["wihg"].ap())
    nc.sync.dma_start(out=w_whhg, in_=dram["whhg"].ap())
    nc.sync.dma_start(out=w_biasg, in_=dram["bias_g"].ap())
    nc.sync.dma_start(out=w_wgen, in_=dram["wgen"].ap())
    nc.sync.dma_start(out=w_bgen, in_=dram["bias_gen"].ap())

    # ---- big activations ----
    an_bf = data.tile([128, 2, BC, T], BF16, name="an_bf")
    ct_sb = data.tile([128, 4, TB], BF16, name="ct_sb")
    lstm_out = data.tile([128, 4, TB], BF16, name="lstm_out")
    xe = [data.tile([128, TB], BF16, tag=f"xe{k}", name=f"xe{k}")
          for k in range(4)]
    xpre = [data.tile([128, 8, TB], BF16, tag=f"xpre{d}", name=f"xpre{d}")
            for d in range(2)]
    gi_emb = data.tile([128, 12, TB], F32, name="gi_emb")
    gi = data.tile([128, 12, TB], BF16, name="gi")
    hg = data.tile([128, 4, TB], BF16, name="hg")
    attn_f32 = data.tile([128, 2, BC, T], F32, name="attn_f32")
    logits_sb = data.tile([NCLASS, TB], F32, name="logits_sb")

    for j in range(4):
        nc.sync.dma_start(out=xe[j], in_=dram["embT"].ap()[:, j, :])

    # constants / states
    ones_col = state.tile([128, 1], F32, name="ones_col")
    ones_row = state.tile([1, 128], F32, name="ones_row")
    zer_rhs = state.tile([128, BC], BF16, name="zer_rhs")
    zer_h4 = state.tile([128, 4, BC], BF16, name="zer_h4")
    lstm_c = state.tile([128, 2, 2, BC], F32, name="lstm_c")
    nc.vector.memset(ones_col, 1.0)
    nc.vector.memset(ones_row, 1.0)
    nc.vector.memset(zer_rhs, 0.0)
    nc.vector.memset(zer_h4, 0.0)
    nc.vector.memset(lstm_c, 0.0)

    # ============ phase 1: normalize A ============
    ps_sum = ps.tile([1, TB], F32, tag="pA", bufs=1, name="ps_sum")
    for kt in range(2):
        nc.tensor.matmul(ps_sum, ones_col,
                         w_A[:, kt].rearrange("p b t -> p (b t)"),
                         start=(kt == 0), stop=(kt == 1))
    recip = ew.tile([1, TB], F32, name="recip")
    nc.vector.reciprocal(recip, ps_sum)
    ps_bc = ps.tile([128, BC, T], F32, tag="pA", bufs=1, name="ps_bc")
    nc.tensor.matmul(ps_bc.rearrange("p b t -> p (b t)"), ones_row, recip,
                     start=True, stop=True)
    for kt in range(2):
        nc.vector.tensor_mul(an_bf[:, kt], w_A[:, kt], ps_bc)
        nc.vector.tensor_mul(attn_f32[:, kt], w_A[:, kt], ps_bc)
    nc.sync.dma_start(out=dram["attn_o"].ap(), in_=attn_f32)
    if phases < 2:
        return

    # ============ phase 2: attention pooling -> ct_sb ============
    for ct in range(4):
        ps_c = ps.tile([128, BC, T], F32, tag="pA", bufs=1, name="ps_c")
        for b in range(BC):
            for kt in range(2):
                nc.tensor.matmul(
                    ps_c[:, b, :],
                    w_feat[:, kt, b, ct * 128:(ct + 1) * 128],
                    an_bf[:, kt, b, :],
                    start=(b == 0 and kt == 0),
                    stop=(b == BC - 1 and kt == 1))
        nc.vector.tensor_copy(
            ct_sb[:, ct].rearrange("p (t b) -> p b t", b=BC), ps_c)
    if phases < 3:
        return

    # ============ phase 3: LSTM x-precompute ============
    for d in range(2):
        for mt in range(8):
            ps_x = ps.tile([128, TB], F32, tag="pX", bufs=2, name="ps_x")
            for kt in range(4):
                nc.tensor.matmul(
                    ps_x, w_wih[d][:, kt, mt * 128:(mt + 1) * 128],
                    ct_sb[:, kt], start=(kt == 0), stop=(kt == 3))
            nc.scalar.activation(xpre[d][:, mt], ps_x, AF.Identity,
                                 bias=w_bias[d][:, mt:mt + 1])
    if phases < 4:
        return

    # ============ phase 4: BiLSTM recurrence ============
    # gate order (host-permuted): [i, f, o, g]; mt 0..5 sigmoid, 6..7 tanh
    # sig-part matmuls first so Sigmoid overlaps the tanh-part matmul burst
    for s in range(T):
        tcs = [s * BC, (T - 1 - s) * BC]          # col offset per dir
        ps_s = ps.tile([128, 2, 6, BC], F32, tag="psg", bufs=3, name="ps_s")
        ps_t = ps.tile([128, 2, 2, BC], F32, tag="psn", bufs=2, name="ps_t")

        def rhs_of(d, kt):
            if s == 0:
                return zer_rhs
            tp = (s - 1) if d == 0 else (T - s)
            return lstm_out[:, 2 * d + kt, tp * BC:(tp + 1) * BC]

        for d in range(2):
            nc.tensor.matmul(ps_s[:, d], w_I,
                             xpre[d][:, 0:6, tcs[d]:tcs[d] + BC],
                             start=(d == 0), stop=False)
        for d in range(2):
            for kt in range(2):
                rhs = rhs_of(d, kt)
                for mt in range(6):
                    nc.tensor.matmul(
                        ps_s[:, d, mt],
                        w_whh[d][:, kt, mt * 128:(mt + 1) * 128],
                        rhs, start=False,
                        stop=(d == 1 and kt == 1 and mt == 5))
        for d in range(2):
            nc.tensor.matmul(ps_t[:, d], w_I,
                             xpre[d][:, 6:8, tcs[d]:tcs[d] + BC],
                             start=(d == 0), stop=False)
        for d in range(2):
            for kt in range(2):
                rhs = rhs_of(d, kt)
                for mt in range(6, 8):
                    nc.tensor.matmul(
                        ps_t[:, d, mt - 6],
                        w_whh[d][:, kt, mt * 128:(mt + 1) * 128],
                        rhs, start=False,
                        stop=(d == 1 and kt == 1 and mt == 7))
        sig = ew.tile([128, 2, 6, BC], F32, tag="sig", name="sig")
        tg = ew.tile([128, 2, 2, BC], F32, tag="tg", name="tg")
        nc.scalar.activation(sig, ps_s, AF.Sigmoid)
        nc.scalar.activation(tg, ps_t, AF.Tanh)
        tmp = ew.tile([128, 2, 2, BC], F32, tag="tmp", name="tmp")
        nc.vector.tensor_mul(lstm_c, lstm_c, sig[:, :, 2:4])
        nc.vector.tensor_mul(tmp, sig[:, :, 0:2], tg)
        nc.vector.tensor_add(lstm_c, lstm_c, tmp)
        tanhc = ew.tile([128, 2, 2, BC], F32, tag="tanhc", name="tanhc")
        nc.scalar.activation(tanhc, lstm_c, AF.Tanh)
        for d in range(2):
            nc.vector.tensor_mul(
                lstm_out[:, 2 * d:2 * d + 2, tcs[d]:tcs[d] + BC],
                sig[:, d, 4:6], tanhc[:, d])
    if phases < 5:
        return

    # ============ phase 5: GRU gi precompute ============
    # emb half: independent of the LSTM -> fills PE gaps during phase 4
    for mt in range(12):
        ps_ge = ps.tile([128, TB], F32, tag="pX", bufs=2, name="ps_ge")
        for kt in range(4):
            nc.tensor.matmul(ps_ge, w_wihg[:, 4 + kt, mt * 128:(mt + 1) * 128],
                             xe[kt], start=(kt == 0), stop=(kt == 3))
        nc.scalar.activation(gi_emb[:, mt], ps_ge, AF.Identity,
                             bias=w_biasg[:, mt:mt + 1])
    # lstm half + combine
    for mt in range(12):
        ps_gi = ps.tile([128, TB], F32, tag="pX", bufs=2, name="ps_gi")
        for kt in range(4):
            nc.tensor.matmul(ps_gi, w_wihg[:, kt, mt * 128:(mt + 1) * 128],
                             lstm_out[:, kt], start=(kt == 0), stop=(kt == 3))
        nc.vector.scalar_tensor_tensor(
            out=gi[:, mt], in0=ps_gi, scalar=0.0, in1=gi_emb[:, mt],
            op0=mybir.AluOpType.bypass, op1=mybir.AluOpType.add)
    if phases < 6:
        return

    # ============ phase 6: GRU recurrence ============
    # weight rows: r = 0:512, z = 512:1024, n = 1024:1536
    # block order r -> n -> z so the r/n consumers overlap the z matmuls
    for t in range(T):
        tc0, tc1 = t * BC, (t + 1) * BC
        hprev = zer_h4 if t == 0 else hg[:, :, tc0 - BC:tc0]
        ps_r = ps.tile([128, 4, BC], F32, tag="psg", bufs=3, name="ps_r")
        ps_z = ps.tile([128, 4, BC], F32, tag="psg", bufs=3, name="ps_z")
        ps_n = ps.tile([128, 4, BC], F32, tag="psn", bufs=2, name="ps_n")

        def hrhs(kt):
            return zer_rhs if t == 0 else hg[:, kt, tc0 - BC:tc0]

        nc.tensor.matmul(ps_r, w_I, gi[:, 0:4, tc0:tc1],
                         start=True, stop=False)
        for kt in range(4):
            for mt in range(4):
                nc.tensor.matmul(
                    ps_r[:, mt], w_whhg[:, kt, mt * 128:(mt + 1) * 128],
                    hrhs(kt), start=False, stop=(kt == 3 and mt == 3))
        nc.tensor.matmul(ps_n, w_bhhn, w_emt, start=True, stop=False)
        for kt in range(4):
            for mt in range(4):
                nc.tensor.matmul(
                    ps_n[:, mt],
                    w_whhg[:, kt, 1024 + mt * 128:1024 + (mt + 1) * 128],
                    hrhs(kt), start=False, stop=(kt == 3 and mt == 3))
        nc.tensor.matmul(ps_z, w_I, gi[:, 4:8, tc0:tc1],
                         start=True, stop=False)
        for kt in range(4):
            for mt in range(4):
                nc.tensor.matmul(
                    ps_z[:, mt],
                    w_whhg[:, kt, 512 + mt * 128:512 + (mt + 1) * 128],
                    hrhs(kt), start=False, stop=(kt == 3 and mt == 3))
        r_t = ew.tile([128, 4, BC], F32, tag="r_t", name="r_t")
        z_t = ew.tile([128, 4, BC], F32, tag="z_t", name="z_t")
        zc_t = ew.tile([128, 4, BC], F32, tag="zc_t", name="zc_t")
        nc.scalar.activation(r_t, ps_r, AF.Sigmoid)
        nt = ew.tile([128, 4, BC], F32, tag="nt", name="nt")
        nc.vector.tensor_mul(nt, ps_n, r_t)
        nc.vector.tensor_add(nt, nt, gi[:, 8:12, tc0:tc1])
        nc.scalar.activation(zc_t, ps_z, AF.Sigmoid, scale=-1.0)
        nc.scalar.activation(z_t, ps_z, AF.Sigmoid)
        ntt = ew.tile([128, 4, BC], F32, tag="ntt", name="ntt")
        nc.scalar.activation(ntt, nt, AF.Tanh)
        p1 = ew.tile([128, 4, BC], F32, tag="p1", name="p1")
        nc.vector.tensor_mul(p1, hprev, z_t)
        nc.vector.tensor_mul(ntt, ntt, zc_t)
        nc.vector.tensor_add(hg[:, :, tc0:tc1], p1, ntt)
    if phases < 7:
        return

    # ============ phase 7: generator ============
    ps_l = ps.tile([NCLASS, TB], F32, tag="pX", bufs=2, name="ps_l")
    for kt in range(4):
        nc.tensor.matmul(ps_l, w_wgen[:, kt], hg[:, kt],
                         start=(kt == 0), stop=(kt == 3))
    nc.scalar.activation(logits_sb, ps_l, AF.Identity, bias=w_bgen)
    nc.sync.dma_start(out=dram["logits_o"].ap(), in_=logits_sb)


def prep_inputs(inputs):
    """Host-side: shard + lay out all tensors exactly as the SBUF tiles expect."""
    feature = np.asarray(inputs["feature"], np.float32)
    A = np.asarray(inputs["A"], np.float32)
    text = np.asarray(inputs["text"])
    char_emb = np.asarray(inputs["char_emb"], np.float32)

    def as_bf(x):
        return np.ascontiguousarray(x.astype(BF16_NP))

    # LSTM gate permutation i,f,g,o -> i,f,o,g
    perm = np.concatenate([np.arange(0, 512), np.arange(768, 1024),
                           np.arange(512, 768)])

    def lhsT_tiles(w):  # (K, M) -> (128, K//128, M)
        K, M = w.shape
        return np.ascontiguousarray(
            w.reshape(K // 128, 128, M).transpose(1, 0, 2))

    def lstm_w(d):
        sfx = "f" if d == 0 else "b"
        wih = np.asarray(inputs[f"W_ih_{sfx}"], np.float32)[perm]
        whh = np.asarray(inputs[f"W_hh_{sfx}"], np.float32)[perm]
        bias = (np.asarray(inputs[f"b_ih_{sfx}"], np.float32)
                + np.asarray(inputs[f"b_hh_{sfx}"], np.float32))[perm]
        return (as_bf(lhsT_tiles(wih.T)), as_bf(lhsT_tiles(whh.T)),
                np.ascontiguousarray(bias.reshape(8, 128).T))

    wihf, whhf, biasf = lstm_w(0)
    wihb, whhb, biasb = lstm_w(1)

    wihg_np = np.asarray(inputs["W_ih_g"], np.float32)
    whhg_np = np.asarray(inputs["W_hh_g"], np.float32)
    bihg = np.asarray(inputs["b_ih_g"], np.float32)
    bhhg = np.asarray(inputs["b_hh_g"], np.float32)
    biasg = bihg + np.concatenate([bhhg[:1024], np.zeros(512, np.float32)])
    bhhn_np = bhhg[1024:]
    bhhn_tiles = np.zeros((128, 128), np.float32)
    bhhn_tiles[:4] = bhhn_np.reshape(4, 128)
    wgen_np = np.asarray(inputs["W_gen"], np.float32)
    bgen = np.asarray(inputs["b_gen"], np.float32)

    emt = np.zeros((128, 4, BC), np.float32)
    for _mt in range(4):
        emt[_mt, _mt, :] = 1.0

    shared = {
        "wih_f": wihf, "wih_b": wihb, "whh_f": whhf, "whh_b": whhb,
        "wihg": as_bf(lhsT_tiles(wihg_np.T)),
        "whhg": as_bf(lhsT_tiles(whhg_np.T)),
        "wgen": as_bf(lhsT_tiles(wgen_np.T)),
        "bias_f": biasf, "bias_b": biasb,
        "bias_g": np.ascontiguousarray(biasg.reshape(12, 128).T),
        "bhhn": as_bf(bhhn_tiles),
        "bias_gen": np.ascontiguousarray(bgen.reshape(NCLASS, 1)),
        "ident": as_bf(np.eye(128, dtype=np.float32)),
        "emt": as_bf(emt),
    }

    prev_tok = np.concatenate(
        [np.zeros((B, 1), text.dtype), text[:, :-1]], axis=1)
    emb = char_emb[prev_tok]                                       # (B,T,512)

    in_maps = []
    for k in range(NCORES):
        sl = slice(k * BC, (k + 1) * BC)
        fk = feature[sl].reshape(BC, NCH, HWS)
        featT = fk.transpose(2, 0, 1).reshape(2, 128, BC, NCH)
        featT = np.ascontiguousarray(featT.transpose(1, 0, 2, 3))
        ak = A[sl].reshape(BC, T, HWS)
        A_tk = ak.transpose(2, 0, 1).reshape(2, 128, BC, T)
        A_tk = np.ascontiguousarray(A_tk.transpose(1, 0, 2, 3))
        ek = emb[sl]
        embT = ek.transpose(2, 1, 0).reshape(4, 128, T * BC)
        embT = np.ascontiguousarray(embT.transpose(1, 0, 2))
        m = dict(shared)
        m["featT"] = as_bf(featT)
        m["A_t"] = A_tk.astype(np.float32)
        m["embT"] = as_bf(embT)
        in_maps.append(m)
    return in_maps


def postprocess(results, text_length):
    logits = np.zeros((T, B, NCLASS), np.float32)
    attns = np.zeros((B, T, HWS), np.float32)
    for k, r in enumerate(results):
        sl = slice(k * BC, (k + 1) * BC)
        lo = r["logits_o"].reshape(NCLASS, T, BC)
        logits[:, sl, :] = lo.transpose(1, 2, 0)
        at = r["attn_o"]
        attns[sl] = at.transpose(2, 3, 1, 0).reshape(BC, T, HWS)
    lens = np.asarray(text_length)
    b_idx = np.repeat(np.arange(B), lens)
    t_idx = np.concatenate([np.arange(l) for l in lens])
    out_res = logits[t_idx, b_idx]
    out_attns = attns[b_idx, t_idx].reshape(-1, 8, 32)
    return out_res, out_attns


_CACHED = {}


def get_program(repeat: int = 0, phases: int = 7):
    key = (repeat, phases)
    if key not in _CACHED:
        _CACHED[key] = build_program(repeat, phases)
    return _CACHED[key]


def kernel(**inputs):
    nc = get_program(0)
    in_maps = prep_inputs(inputs)
    res = run_bass_kernel_spmd(nc, in_maps, core_ids=list(range(NCORES)))
    return postprocess(res.results, inputs["text_length"])


def _pjrt_runner(nc):
    """Persistent jitted SPMD callable (for repeated timed runs)."""
    import jax
    from jax.sharding import Mesh, PartitionSpec
    from jax.experimental.shard_map import shard_map
    from concourse.bass2jax import (_bass_exec_p, install_neuronx_cc_hook,
                                    partition_id_tensor)

    install_neuronx_cc_hook()
    pname = nc.partition_id_tensor.name if nc.partition_id_tensor else None
    in_names, out_names, out_avals, zero_outs = [], [], [], []
    for alloc in nc.m.functions[0].allocations:
        if not isinstance(alloc, mybir.MemoryLocationSet):
            continue
        name = alloc.memorylocations[0].name
        if alloc.kind == "ExternalInput":
            if name != pname:
                in_names.append(name)
        elif alloc.kind == "ExternalOutput":
            out_names.append(name)
            shape = tuple(alloc.tensor_shape)
            dtype = mybir.dt.np(alloc.dtype)
            out_avals.append(jax.core.ShapedArray(shape, dtype))
            zero_outs.append(np.zeros(shape, dtype))
    n_params, n_outs = len(in_names), len(out_avals)
    all_names = list(in_names) + list(out_names)
    if pname is not None:
        all_names.append(pname)

    def _body(*args):
        operands = list(args)
        if pname is not None:
            operands.append(partition_id_tensor())
        return tuple(_bass_exec_p.bind(
            *operands, out_avals=tuple(out_avals), in_names=tuple(all_names),
            out_names=tuple(out_names), lowering_input_output_aliases=(),
            sim_require_finite=True, sim_require_nnan=True, nc=nc))

    devices = jax.devices()[:NCORES]
    mesh = Mesh(np.asarray(devices), ("core",))
    fn = jax.jit(
        shard_map(_body, mesh=mesh,
                  in_specs=(PartitionSpec("core"),) * (n_params + n_outs),
                  out_specs=(PartitionSpec("core"),) * n_outs,
                  check_rep=False),
        keep_unused=True)
    return fn, in_names, zero_outs


def _timed_wall(nc, in_maps, reps):
    import time
    import jax
    from jax.sharding import Mesh, PartitionSpec, NamedSharding
    fn, in_names, zero_outs = _pjrt_runner(nc)
    devices = jax.devices()[:NCORES]
    mesh = Mesh(np.asarray(devices), ("core",))
    shard = NamedSharding(mesh, PartitionSpec("core"))
    concat_in = [jax.device_put(np.concatenate([m[n] for m in in_maps], axis=0),
                                shard) for n in in_names]
    concat_z = [jax.device_put(np.concatenate([z] * NCORES, axis=0), shard)
                for z in zero_outs]
    jax.block_until_ready(concat_in)
    jax.block_until_ready(concat_z)
    jax.block_until_ready(fn(*concat_in, *concat_z))  # warm compile
    best = float("inf")
    for _ in range(reps):
        t0 = time.perf_counter()
        jax.block_until_ready(fn(*concat_in, *concat_z))
        best = min(best, time.perf_counter() - t0)
    return best


def measure_hw_time(inputs, reps=6, r0=1, r1=2049, phases=7):
    """Per-iteration device time via repeat-count delta (cancels dispatch cost)."""
    in_maps = prep_inputs(inputs)
    w0 = _timed_wall(get_program(r0, phases), in_maps, reps)
    w1 = _timed_wall(get_program(r1, phases), in_maps, reps)
    return (w1 - w0) / (r1 - r0) * 1e9
